# revision 10
# baseline (speedup 1.0000x reference)
"""Trainium2 Bass kernel for nn_Decoder_85916525789418 (GRU decoder with
per-scene self-attention), data-parallel over scenes across 8 NeuronCores.

Contract: kernel(**inputs) takes the FULL unsharded inputs (as produced by
reference.setup_inputs) and returns the full (mus, stds) outputs.

v2 design (engine-balanced rewrite of the baseline):
  - batch axis (65536 peds = 4096 scenes x 16) sharded 8 ways at scene
    granularity -> 8192 peds/core, no cross-core communication.
  - host does layout prep only (transpose/slice/pack/dtype casts); all FLOPs
    on device. All linear-layer biases are folded in as an extra ones-row of
    the activations (host-side weight augmentation) or K=1 ones matmuls.
  - GRU gates: r,z accumulate into ONE [128,1024] psum (whh@h + wia@a +
    ident@gx chains per half), read by a single wide Sigmoid on ACT.
    n-gate: psn = whh_n@h + b_hh_n (K=1 ones matmul); tmp = sigm(r)*psn on
    DVE; npre = psn2 + tmp (DVE stt); Tanh over tile-pairs on ACT.
    Update h' = ng + sigm(z)*(h-ng): dd on Pool (pair-wide), m2/h' on DVE.
  - attention: block-diag mask folded INTO the score psum as +30*C@C^T (K=8
    matmul) with a uniform -30 exp bias -> masked exp for free. Softmax
    denominators via ones128 matmul (replicated along partitions), ONE
    reciprocal from PSUM (bf16), R = e * rden (R == attn^T by symmetry of e).
    fc_attn ctx term via AT = (W2 h)^T per group (no h transposes at all):
    psfc = W1@h + sum_g AT_g^T @ R_g + battn.
  - mu/std: per-group [128,4] matmuls + bias K=1, per-timestep exp of std
    cols, one DMA per timestep.
"""

import sys

for _p in ("/opt/trn_rl_repo",):
    if _p not in sys.path:
        sys.path.insert(0, _p)

import numpy as np
import ml_dtypes

import concourse.bass as bass
import concourse.mybir as mybir
from concourse import bacc, tile
from concourse.bass import ts, ds

NCORES = 8
B, PED, H, MLP, ZD, TT, NS, NP = 65536, 16, 128, 256, 32, 12, 6, 2
ZX = MLP + ZD          # 288
ZXA = ZX + 1           # 289 (ones row for bias folding)
G3 = 3 * H             # 384
BL = B // NCORES       # 8192 peds per core
NB = 512               # peds per processing tile
NGRP = NB // 128       # 4 groups of 128 peds per tile

F32 = mybir.dt.float32
F32R = mybir.dt.float32r
BF16 = mybir.dt.bfloat16
FP16 = mybir.dt.float16
AF = mybir.ActivationFunctionType
OP = mybir.AluOpType
BF16NP = ml_dtypes.bfloat16
FP16NP = np.float16


def build_module(bl=BL, t_steps=TT):
    nt = bl // NB
    nc = bacc.Bacc("TRN2", target_bir_lowering=False)

    # ---- DRAM I/O ----
    d_zx0 = nc.dram_tensor("zx0", [128, bl], BF16, kind="ExternalInput")
    d_zx1 = nc.dram_tensor("zx1", [128, bl], BF16, kind="ExternalInput")
    d_zx2 = nc.dram_tensor("zx2", [ZXA - 256, bl], BF16, kind="ExternalInput")
    d_lsT = nc.dram_tensor("lsT", [NS + 1, bl], F32R, kind="ExternalInput")
    d_futT = nc.dram_tensor("futT", [t_steps * NP, bl], FP16, kind="ExternalInput")
    d_whhT = nc.dram_tensor("whhT", [H, G3], FP16, kind="ExternalInput")
    d_wiaT = nc.dram_tensor("wiaT", [NP, G3], FP16, kind="ExternalInput")
    d_wihx0 = nc.dram_tensor("wihx0", [128, G3], BF16, kind="ExternalInput")
    d_wihx1 = nc.dram_tensor("wihx1", [128, G3], BF16, kind="ExternalInput")
    d_wihx2 = nc.dram_tensor("wihx2", [ZXA - 256, G3], BF16, kind="ExternalInput")
    d_wdec0 = nc.dram_tensor("wdec0", [128, H], BF16, kind="ExternalInput")
    d_wdec1 = nc.dram_tensor("wdec1", [128, H], BF16, kind="ExternalInput")
    d_wdec2 = nc.dram_tensor("wdec2", [ZXA - 256, H], BF16, kind="ExternalInput")
    d_wvelT = nc.dram_tensor("wvelT", [NS + 1, NP], F32R, kind="ExternalInput")
    d_wat1 = nc.dram_tensor("wat1", [H, H], FP16, kind="ExternalInput")
    d_wat2 = nc.dram_tensor("wat2", [H, H], FP16, kind="ExternalInput")
    d_wms = nc.dram_tensor("wms", [H, 2 * NP], FP16, kind="ExternalInput")
    d_bms = nc.dram_tensor("bms", [1, 2 * NP], FP16, kind="ExternalInput")
    d_ident = nc.dram_tensor("ident", [128, 128], FP16, kind="ExternalInput")
    d_ones128 = nc.dram_tensor("ones128", [128, 128], BF16, kind="ExternalInput")
    d_ct8 = nc.dram_tensor("ct8", [8, 128], FP16, kind="ExternalInput")
    d_ct8x30 = nc.dram_tensor("ct8x30", [8, 128], FP16, kind="ExternalInput")
    d_rows = nc.dram_tensor("rows", [1, 512 + 3 * 128 + 4], FP16,
                            kind="ExternalInput")  # ones512|bhhn|battn|onescol|bms... packed
    d_m30 = nc.dram_tensor("m30", [128, 1], F32, kind="ExternalInput")
    d_zero = nc.dram_tensor("zero", [128, 1], F32, kind="ExternalInput")
    # out[t, g, c, p]: c in (mu0, mu1, std0, std1), g = group id, p = ped in group
    d_out = nc.dram_tensor("outT", [t_steps, bl // 128, 4, 128], F32,
                           kind="ExternalOutput")

    with tile.TileContext(nc) as tc:
        with (
            tc.tile_pool(name="singles", bufs=1) as singles,
            tc.tile_pool(name="zxp", bufs=2) as zxp,
            tc.tile_pool(name="ga", bufs=4) as ga,
            tc.tile_pool(name="gp", bufs=3) as gp,
            tc.tile_pool(name="ab", bufs=4) as ab_pool,
            tc.tile_pool(name="psum", bufs=1, space="PSUM") as psum,
        ):
            # ---- persistent SBUF state ----
            hT = singles.tile([128, bl], FP16)
            gx0 = singles.tile([128, bl], FP16)
            gx1 = singles.tile([128, bl], FP16)
            gx2 = singles.tile([128, bl], FP16)
            a_bufs = [singles.tile([NP, bl], FP16, name=f"acur{j}") for j in range(2)]
            msbufs = [singles.tile([128, 16 * (bl // NB)], F32, name=f"msb{j}")
                      for j in range(2)]

            whhT = singles.tile([H, G3], FP16)
            wiaT = singles.tile([NP, G3], FP16)
            wihx0 = singles.tile([128, G3], BF16)
            wihx1 = singles.tile([128, G3], BF16)
            wihx2 = singles.tile([ZXA - 256, G3], BF16)
            wdec0 = singles.tile([128, H], BF16)
            wdec1 = singles.tile([128, H], BF16)
            wdec2 = singles.tile([ZXA - 256, H], BF16)
            wvelT = singles.tile([NS + 1, NP], F32R)
            wat1 = singles.tile([H, H], FP16)
            wat2 = singles.tile([H, H], FP16)
            wms = singles.tile([H, 2 * NP], FP16)
            bms = singles.tile([1, 2 * NP], FP16)
            ident = singles.tile([128, 128], FP16)
            ones128 = singles.tile([128, 128], BF16)
            ct8 = singles.tile([8, 128], FP16)
            ct8x30 = singles.tile([8, 128], FP16)
            rows = singles.tile([1, 512 + 3 * 128 + 4], FP16)
            m30 = singles.tile([128, 1], F32)
            zerocol = singles.tile([128, 1], F32)

            for dst, src in [
                (whhT, d_whhT), (wiaT, d_wiaT),
                (wihx0, d_wihx0), (wihx1, d_wihx1), (wihx2, d_wihx2),
                (wdec0, d_wdec0), (wdec1, d_wdec1), (wdec2, d_wdec2),
                (wvelT, d_wvelT), (wat1, d_wat1), (wat2, d_wat2),
                (wms, d_wms), (bms, d_bms), (ident, d_ident),
                (ones128, d_ones128), (ct8, d_ct8), (ct8x30, d_ct8x30),
                (rows, d_rows), (m30, d_m30), (zerocol, d_zero),
            ]:
                nc.sync.dma_start(dst[:], src[:])

            ones512 = rows[:, 0:512]
            bhhn_row = rows[:, 512:640]
            battn_row = rows[:, 640:768]
            ones128r = rows[:, 768:896]

            # ---- prologue: gx (gates' zx part + biases), h0, a0 ----
            for i in range(nt):
                sl = ts(i, NB)
                z0 = zxp.tile([128, NB], BF16, tag="z0")
                z1 = zxp.tile([128, NB], BF16, tag="z1")
                z2 = zxp.tile([ZXA - 256, NB], BF16, tag="z2")
                nc.sync.dma_start(z0[:], d_zx0[:, sl])
                nc.sync.dma_start(z1[:], d_zx1[:, sl])
                nc.sync.dma_start(z2[:], d_zx2[:, sl])
                for oc, gxs in enumerate((gx0, gx1, gx2)):
                    ps = psum.tile([128, NB], F32, tag=["pn", "pn2", "psc"][oc])
                    nc.tensor.matmul(ps[:], wihx0[:, ts(oc, 128)], z0[:],
                                     start=True, stop=False)
                    nc.tensor.matmul(ps[:], wihx1[:, ts(oc, 128)], z1[:],
                                     start=False, stop=False)
                    nc.tensor.matmul(ps[:], wihx2[:, ts(oc, 128)], z2[:],
                                     start=False, stop=True)
                    if oc == 0:
                        nc.scalar.copy(gxs[:, sl], ps[:])
                    elif oc == 1:
                        nc.vector.tensor_copy(gxs[:, sl], ps[:])
                    else:
                        nc.scalar.copy(gxs[:, sl], ps[:])
                # h0
                psh = psum.tile([128, NB], F32, tag="pfc")
                nc.tensor.matmul(psh[:], wdec0[:], z0[:], start=True, stop=False)
                nc.tensor.matmul(psh[:], wdec1[:], z1[:], start=False, stop=False)
                nc.tensor.matmul(psh[:], wdec2[:], z2[:], start=False, stop=True)
                nc.vector.tensor_copy(hT[:, sl], psh[:])
                # a0 = [last_state; 1] @ wvel_aug
                lst = zxp.tile([NS + 1, NB], F32R, tag="ls")
                nc.sync.dma_start(lst[:], d_lsT[:, sl])
                psa = psum.tile([128, NB], F32, tag="pden")
                nc.tensor.matmul(psa[0:NP, :], wvelT[:], lst[:], start=True,
                                 stop=True)
                nc.scalar.copy(a_bufs[0][:, sl], psa[0:NP, :])

            # ---- time loop ----
            for t in range(t_steps):
                a_cur = a_bufs[t % 2]
                msbuf = msbufs[t % 2]
                if t > 0:
                    nc.sync.dma_start(a_cur[:], d_futT[ds(NP * (t - 1), NP), :])

                # --- phase A: GRU ---
                ngp = None
                ddp = None
                for i in range(nt):
                    sl = ts(i, NB)
                    psrz = psum.tile([128, 2 * NB], F32, tag="prz")
                    psn = psum.tile([128, NB], F32, tag="pn")
                    psn2 = psum.tile([128, NB], F32, tag="pn2")
                    for c, gxs in ((0, gx0), (1, gx1)):
                        reg = psrz[:, ts(c, NB)]
                        nc.tensor.matmul(reg, whhT[:, ts(c, 128)], hT[:, sl],
                                         start=True, stop=False)
                        nc.tensor.matmul(reg, wiaT[:, ts(c, 128)], a_cur[:, sl],
                                         start=False, stop=False)
                        nc.tensor.matmul(reg, ident[:], gxs[:, sl],
                                         start=False, stop=True)
                    # psn = whh_n @ h + b_hh_n
                    nc.tensor.matmul(psn[:], whhT[:, 256:384], hT[:, sl],
                                     start=True, stop=False)
                    nc.tensor.matmul(psn[:], bhhn_row, ones512,
                                     start=False, stop=True)
                    # psn2 = wia_n @ a + gx2 (incl. b_ih_n)
                    nc.tensor.matmul(psn2[:], wiaT[:, 256:384], a_cur[:, sl],
                                     start=True, stop=False)
                    nc.tensor.matmul(psn2[:], ident[:], gx2[:, sl],
                                     start=False, stop=True)

                    # wrz = tanh(0.5 * gates); sigm(x) = 0.5*wrz + 0.5
                    srz = ga.tile([128, 2 * NB], FP16, tag="srz", bufs=5)
                    tmp = ga.tile([128, NB], FP16, tag="tmp")
                    if i % 2 == 0:
                        ngp = ga.tile([128, 2 * NB], FP16, tag="ngp", bufs=3)
                        ddp = ga.tile([128, 2 * NB], FP16, tag="ddp", bufs=3)
                    npre_half = ngp[:, ts(i % 2, NB)]

                    nc.scalar.activation(srz[:], psrz[:], AF.Tanh, scale=0.5)
                    # tmp = sigm(r)*(whh_n h + b_hh_n) = (wr+1)*psn
                    # (whh_n and b_hh_n are pre-scaled by 0.5 on the host)
                    nc.vector.scalar_tensor_tensor(tmp[:], srz[:, 0:NB], 1.0,
                                                   psn[:], OP.add, OP.mult)
                    # npre = psn2 + tmp  (write into pair buffer half)
                    nc.vector.scalar_tensor_tensor(npre_half, psn2[:], 0.0,
                                                   tmp[:], OP.add, OP.add)
                    if i % 2 == 1:
                        psl = ds((i - 1) * NB, 2 * NB)
                        # ng = tanh(npre) in-place over the pair
                        nc.scalar.activation(ngp[:], ngp[:], AF.Tanh)
                        # dd = h - ng over the pair (Pool)
                        nc.gpsimd.tensor_sub(ddp[:], hT[:, psl], ngp[:])
                        for j in (i - 1, i):
                            jl = ts(j, NB)
                            jh = ts(j % 2, NB)
                            srz_j = srz_prev if j < i else srz
                            zgj = ga.tile([128, NB], FP16, tag="zg", bufs=4)
                            m2j = ga.tile([128, NB], FP16, tag="m2", bufs=4)
                            nc.vector.tensor_scalar(
                                zgj[:], srz_j[:, NB:2 * NB], 0.5, 0.5,
                                OP.mult, OP.add)
                            nc.vector.tensor_tensor(
                                m2j[:], zgj[:], ddp[:, jh], OP.mult)
                            nc.vector.tensor_tensor(
                                hT[:, jl], ngp[:, jh], m2j[:], OP.add)
                    srz_prev = srz

                # --- phase B: attention + fc + outputs ---
                for i in range(nt):
                    sl = ts(i, NB)
                    pssc = psum.tile([128, NB], F32, tag="psc")
                    psden = psum.tile([128, NB], F32, tag="pden")
                    psAT = psum.tile([128, NB], F32, tag="pat")
                    psfc = psum.tile([128, NB], F32, tag="pfc")
                    expv = ab_pool.tile([128, NB], BF16, tag="expv", bufs=5)
                    rden = ab_pool.tile([128, NB], BF16, tag="rden", bufs=5)
                    Rm = ab_pool.tile([128, NB], BF16, tag="Rm", bufs=5)
                    ATc = ab_pool.tile([128, NB], BF16, tag="ATc", bufs=5)

                    for g in range(NGRP):
                        go = ts(g, 128)
                        gb = ds(i * NB + g * 128, 128)
                        # scores + in-block +30 (mask bias)
                        nc.tensor.matmul(pssc[:, go], hT[:, gb], hT[:, gb],
                                         start=True, stop=False)
                        nc.tensor.matmul(pssc[:, go], ct8[:], ct8x30[:],
                                         start=False, stop=True)
                        # AT_g = (W2 h_g)^T = h_g^T @ W2^T
                        nc.tensor.matmul(psAT[:, go], hT[:, gb], wat2[:],
                                         start=True, stop=True)
                    # e = exp(S - 30), masked for free
                    nc.scalar.activation(expv[:], pssc[:], AF.Exp,
                                         bias=m30[:, 0:1])
                    # den replicated along partitions (per 128-group)
                    for g in range(NGRP):
                        go = ts(g, 128)
                        nc.tensor.matmul(psden[:, go], ones128[:], expv[:, go],
                                         start=True, stop=True)
                    with nc.allow_low_precision(reason="softmax denom bf16"):
                        nc.vector.reciprocal(rden[:], psden[:])
                    # R = e * (1/den) == attn^T (e is symmetric per group)
                    nc.vector.tensor_tensor(Rm[:], expv[:], rden[:], OP.mult)
                    # ATc copy psum->sbuf
                    nc.scalar.copy(ATc[:], psAT[:])
                    # psfc = W1 h + sum_g AT_g^T R_g + battn
                    nc.tensor.matmul(psfc[:], wat1[:], hT[:, sl],
                                     start=True, stop=False)
                    for g in range(NGRP):
                        go = ts(g, 128)
                        nc.tensor.matmul(psfc[:, go], ATc[:, go], Rm[:, go],
                                         start=False, stop=False,
                                         skip_group_check=True)
                    nc.tensor.matmul(psfc[:], battn_row, ones512,
                                     start=False, stop=True)
                    if i % 2 == 0:
                        nc.scalar.copy(hT[:, sl], psfc[:])
                    else:
                        nc.vector.tensor_copy(hT[:, sl], psfc[:])
                    # mu/std ([128 peds, 4] per group) + bias via K=1
                    psms = psum.tile([128, NB], F32, tag="pden")
                    for g in range(NGRP):
                        gb = ds(i * NB + g * 128, 128)
                        nc.tensor.matmul(psms[:, ts(g, 4)], hT[:, gb], wms[:],
                                         start=True, stop=False)
                        nc.tensor.matmul(psms[:, ts(g, 4)], ones128r, bms[:],
                                         start=False, stop=True)
                    nc.vector.tensor_copy(msbuf[:, ds(16 * i, 16)],
                                          psms[:, 0:16])

                # std cols -> exp(0.5*x) in place, then one DMA per timestep
                nc.scalar.activation(
                    msbuf[:].rearrange("p (i c) -> p i c", c=4)[:, :, 2:4],
                    msbuf[:].rearrange("p (i c) -> p i c", c=4)[:, :, 2:4],
                    AF.Exp, bias=zerocol[:, 0:1], scale=0.5)
                nc.sync.dma_start(
                    d_out[t].rearrange("g c p -> p (g c)"), msbuf[:])

    nc.compile()
    return nc


def _host_pack(inputs, bl=BL, t_steps=TT, ncores=NCORES):
    """Slice + lay out the full inputs into per-core in_maps (layout prep only)."""
    f32 = np.float32
    enc = np.asarray(inputs["enc_h_feat"], f32)
    zz = np.asarray(inputs["z"], f32)
    ls = np.asarray(inputs["last_state"], f32)
    fut = np.asarray(inputs["fut_state"], f32)
    W_dec = np.asarray(inputs["W_dec"], f32); b_dec = np.asarray(inputs["b_dec"], f32)
    W_vel = np.asarray(inputs["W_vel"], f32); b_vel = np.asarray(inputs["b_vel"], f32)
    W_ih = np.asarray(inputs["W_ih"], f32); b_ih = np.asarray(inputs["b_ih"], f32)
    W_hh = np.asarray(inputs["W_hh"], f32); b_hh = np.asarray(inputs["b_hh"], f32)
    W_attn = np.asarray(inputs["W_attn"], f32); b_attn = np.asarray(inputs["b_attn"], f32)
    W_mu = np.asarray(inputs["W_mu"], f32); b_mu = np.asarray(inputs["b_mu"], f32)
    W_std = np.asarray(inputs["W_std"], f32); b_std = np.asarray(inputs["b_std"], f32)

    Bfull = enc.shape[0]
    onescol = np.ones((Bfull, 1), f32)
    zxT = np.ascontiguousarray(
        np.concatenate([enc, zz, onescol], axis=1).T).astype(BF16NP)  # [289, B]
    lsT = np.ascontiguousarray(
        np.concatenate([ls, onescol], axis=1).T)                      # [7, B]
    futT = np.ascontiguousarray(fut.transpose(0, 2, 1)).reshape(t_steps * NP, -1)
    futT = futT.astype(FP16NP)

    # n-gate hh-side pre-scaled by 0.5 so tmp = (wr+1)*psn == sigm(r)*nh
    # (r,z sigmoids use the ACT op's own scale=0.5 tanh trick)
    whh_s = W_hh.T.copy()
    whh_s[:, 256:384] *= 0.5
    whhT = np.ascontiguousarray(whh_s).astype(FP16NP)                 # [128, 384]
    wiaT = np.ascontiguousarray(W_ih[:, ZX:].T).astype(FP16NP)        # [2, 384]
    # bias row: b_ih (all gates) + b_hh for r,z only (n handled via K=1 row)
    bias_gates = b_ih.copy()
    bias_gates[0:256] += b_hh[0:256]
    wihxT = np.concatenate([W_ih[:, :ZX].T, bias_gates.reshape(1, G3)],
                           axis=0).astype(BF16NP)                     # [289, 384]
    wdecT = np.concatenate([W_dec.T, b_dec.reshape(1, H)],
                           axis=0).astype(BF16NP)                     # [289, 128]
    wvelT = np.concatenate([W_vel.T, b_vel.reshape(1, NP)], axis=0)   # [7, 2]
    wat1 = np.ascontiguousarray(W_attn.T[0:128]).astype(FP16NP)       # [128, 128]
    wat2 = np.ascontiguousarray(W_attn.T[128:256]).astype(FP16NP)     # [128, 128]
    wms = np.ascontiguousarray(
        np.concatenate([W_mu, W_std], axis=0).T).astype(FP16NP)       # [128, 4]
    bms = np.concatenate([b_mu, b_std]).reshape(1, 4).astype(FP16NP)
    ident = np.eye(128, dtype=FP16NP)
    ones128 = np.ones((128, 128), BF16NP)
    # C [128, 8]: scene-block indicator; rows of C^T
    C = np.kron(np.eye(8, dtype=f32), np.ones((16, 1), f32))          # [128, 8]
    ct8 = np.ascontiguousarray(C.T).astype(FP16NP)                    # [8, 128]
    ct8x30 = (30.0 * ct8.astype(f32)).astype(FP16NP)
    rows = np.zeros((1, 512 + 3 * 128 + 4), FP16NP)
    rows[0, 0:512] = 1.0
    rows[0, 512:640] = (0.5 * b_hh[256:384]).astype(FP16NP)
    rows[0, 640:768] = b_attn.astype(FP16NP)
    rows[0, 768:896] = 1.0
    m30 = np.full((128, 1), -30.0, f32)
    zero = np.zeros((128, 1), f32)

    shared = dict(whhT=whhT, wiaT=wiaT, wihx0=wihxT[0:128], wihx1=wihxT[128:256],
                  wihx2=wihxT[256:ZXA], wdec0=wdecT[0:128], wdec1=wdecT[128:256],
                  wdec2=wdecT[256:ZXA], wvelT=wvelT, wat1=wat1, wat2=wat2,
                  wms=wms, bms=bms, ident=ident, ones128=ones128, ct8=ct8,
                  ct8x30=ct8x30, rows=rows, m30=m30, zero=zero)
    in_maps = []
    for c in range(ncores):
        sl = slice(c * bl, (c + 1) * bl)
        m = dict(shared)
        m["zx0"] = np.ascontiguousarray(zxT[0:128, sl])
        m["zx1"] = np.ascontiguousarray(zxT[128:256, sl])
        m["zx2"] = np.ascontiguousarray(zxT[256:ZXA, sl])
        m["lsT"] = np.ascontiguousarray(lsT[:, sl])
        m["futT"] = np.ascontiguousarray(futT[:, sl])
        in_maps.append(m)
    return in_maps


def _assemble(results, bl=BL, t_steps=TT):
    outs = np.concatenate([r["outT"] for r in results], axis=1)  # [T, B/128, 4, 128]
    o = outs.transpose(0, 1, 3, 2).reshape(t_steps, -1, 4)       # [T, B, 4]
    mus = np.ascontiguousarray(o[:, :, 0:2])
    stds = np.ascontiguousarray(o[:, :, 2:4])
    return mus, stds


_NC_CACHE = {}


def run_kernel(inputs, trace=False, **kw):
    from concourse.bass_utils import run_bass_kernel_spmd
    key = "full"
    if key not in _NC_CACHE:
        _NC_CACHE[key] = build_module()
    nc = _NC_CACHE[key]
    in_maps = _host_pack(inputs)
    res = run_bass_kernel_spmd(nc, in_maps, core_ids=list(range(NCORES)),
                               trace=trace, **kw)
    mus, stds = _assemble(res.results)
    return mus, stds, res


def kernel(**inputs):
    mus, stds, _ = run_kernel(inputs)
    return mus, stds


if __name__ == "__main__":
    pass
